# revision 4
# baseline (speedup 1.0000x reference)
"""nn_CAM_Module kernel for 8 Trainium2 NeuronCores (Bass/Tile).

Contract: kernel(**inputs) takes the FULL inputs (x: [16, 512, 64, 64] fp32,
gamma: [1] fp32) and returns the FULL output, sharding batch B=16 across the
8 cores (2 samples per core, gamma replicated) — per the data-parallel
sharding: every op is a per-sample bmm, no cross-core communication.

Per-sample computation (C=512 channels, N=H*W=4096):
  energy = xf @ xf.T                          (C,C), contraction over N on PE
  m_i    = min_j energy[i,j]                  (softmax(max-e) == softmax(m-e))
  P_ij   = exp(m_i - energy_ij), S_i = sum_j  (ACT, fused row-sum)
  out    = diag(1/S) @ (P @ xf)               (PE; P^T tiles via PE transpose)
  y      = gamma * out + x                    (fused DVE mult-add)

Layouts per core (P=128 partitions):
  xf   [128, 4, 4096] fp32   channel blocks on partitions (DMA from DRAM)
  xfc  [128, 4, 4096] mm_dt  low-precision cast (matmul operand)
  xfT  [128, 32, 512] mm_dt  spatial chunks on partitions (PE transposes)
  Pmat [128, 4, 512]  mm_dt  attention numerator, rows i
  PT   [128, 4, 512]  mm_dt  P^T tiles (PE transposes), matmul2 stationary
"""

import os
from contextlib import ExitStack

import numpy as np

B, C, H, W = 16, 512, 64, 64
N = H * W
N_CORES = 8
BPC = B // N_CORES
P = 128

MM_DT_NAME = os.environ.get("CAM_MM_DT", "bf16")

LAST_EXEC_TIME_NS = None
LAST_TRACE = None
LAST_PROFILE_JSON = None
_CACHE = {}


def _build(mm_dt_name):
    import concourse.mybir as mybir
    import concourse.tile as tile
    from concourse import bacc
    from concourse.masks import make_identity

    F32 = mybir.dt.float32
    mm_dt = {
        "bf16": mybir.dt.bfloat16,
        "fp8": mybir.dt.float8e4,
        "f32": F32,
    }[mm_dt_name]
    DR = mm_dt in (mybir.dt.float8e4, mybir.dt.float8e5)

    CB = C // P          # 4 channel blocks
    KB = N // P          # 32 spatial chunks
    NCH_SZ = 512
    NCH = N // NCH_SZ    # 8 output column chunks

    nc = bacc.Bacc(None, target_bir_lowering=False, debug=False)
    x = nc.dram_tensor("x", [BPC, C, N], F32, kind="ExternalInput")
    gamma = nc.dram_tensor("gamma", [1], F32, kind="ExternalInput")
    y = nc.dram_tensor("y", [BPC, C, N], F32, kind="ExternalOutput")

    with ExitStack() as ctx:
        tc = ctx.enter_context(tile.TileContext(nc))
        singles = ctx.enter_context(tc.tile_pool(name="singles", bufs=1))
        xf_pool = ctx.enter_context(tc.tile_pool(name="xf", bufs=1))
        xfc_pool = ctx.enter_context(tc.tile_pool(name="xfc", bufs=1))
        xfT_pool = ctx.enter_context(tc.tile_pool(name="xfT", bufs=1))
        pmat_pool = ctx.enter_context(tc.tile_pool(name="pmat", bufs=2))
        pt_pool = ctx.enter_context(tc.tile_pool(name="pt", bufs=2))
        small = ctx.enter_context(tc.tile_pool(name="small", bufs=16))
        yt_pool = ctx.enter_context(tc.tile_pool(name="yt", bufs=4))
        eps_pool = ctx.enter_context(tc.tile_pool(name="eps", bufs=2, space="PSUM"))
        tps_pool = ctx.enter_context(tc.tile_pool(name="tps", bufs=3, space="PSUM"))
        ops_pool = ctx.enter_context(tc.tile_pool(name="ops", bufs=3, space="PSUM"))

        ident = singles.tile([P, P], mm_dt)
        make_identity(nc, ident)
        gamma_sb = singles.tile([P, 1], F32)
        nc.sync.dma_start(gamma_sb[:], gamma[:].to_broadcast((P, 1)))

        for b in range(BPC):
            xf = xf_pool.tile([P, CB, N], F32, tag="xf")
            for cb in range(CB):
                nc.sync.dma_start(xf[:, cb, :], x[b, cb * P : (cb + 1) * P, :])

            xfc = xfc_pool.tile([P, CB, N], mm_dt, tag="xfc")
            for cb in range(CB):
                if cb % 2 == 0:
                    nc.vector.tensor_copy(out=xfc[:, cb, :], in_=xf[:, cb, :])
                else:
                    nc.scalar.copy(out=xfc[:, cb, :], in_=xf[:, cb, :])

            xfT = xfT_pool.tile([P, KB, C], mm_dt, tag="xfT")
            for k in range(KB):
                tps = tps_pool.tile([P, CB, P], mm_dt, tag="tps")
                for cb in range(CB):
                    nc.tensor.transpose(
                        tps[:, cb, :], xfc[:, cb, k * P : (k + 1) * P], ident
                    )
                if k % 2 == 0:
                    nc.vector.tensor_copy(out=xfT[:, k, :], in_=tps[:])
                else:
                    nc.scalar.copy(out=xfT[:, k, :], in_=tps[:])

            Pmat = pmat_pool.tile([P, CB, C], mm_dt, tag="pmat")
            rS = small.tile([P, CB], F32, tag="rS")
            for cb in range(CB):
                e_ps = eps_pool.tile([P, C], F32, tag="eps")
                if DR:
                    for k in range(0, KB, 2):
                        nc.tensor.matmul(
                            e_ps[:],
                            xfT[:, k : k + 2, cb * P : (cb + 1) * P],
                            xfT[:, k : k + 2, :],
                            start=(k == 0),
                            stop=(k + 2 >= KB),
                            perf_mode=mybir.MatmulPerfMode.DoubleRow,
                        )
                else:
                    for k in range(KB):
                        nc.tensor.matmul(
                            e_ps[:],
                            xfT[:, k, cb * P : (cb + 1) * P],
                            xfT[:, k, :],
                            start=(k == 0),
                            stop=(k == KB - 1),
                        )
                m = small.tile([P, 1], F32, tag="m")
                nc.vector.tensor_reduce(
                    out=m[:], in_=e_ps[:], axis=mybir.AxisListType.X,
                    op=mybir.AluOpType.min,
                )
                S = small.tile([P, 1], F32, tag="S")
                nc.scalar.activation(
                    out=Pmat[:, cb, :],
                    in_=e_ps[:],
                    func=mybir.ActivationFunctionType.Exp,
                    bias=m[:],
                    scale=-1.0,
                    accum_out=S[:],
                )
                nc.vector.reciprocal(out=rS[:, cb : cb + 1], in_=S[:])

            beta = small.tile([P, CB], F32, tag="beta")
            nc.vector.tensor_tensor(
                out=beta[:],
                in0=rS[:],
                in1=gamma_sb[:].to_broadcast((P, CB)),
                op=mybir.AluOpType.mult,
            )

            PT = pt_pool.tile([P, CB, C], mm_dt, tag="pt")
            for cb in range(CB):
                tps = tps_pool.tile([P, CB, P], mm_dt, tag="tps")
                for ob in range(CB):
                    nc.tensor.transpose(
                        tps[:, ob, :], Pmat[:, ob, cb * P : (cb + 1) * P], ident
                    )
                if cb % 2 == 0:
                    nc.vector.tensor_copy(out=PT[:, cb, :], in_=tps[:])
                else:
                    nc.scalar.copy(out=PT[:, cb, :], in_=tps[:])

            for ob in range(CB):
                for nh in range(NCH):
                    nsl = slice(nh * NCH_SZ, (nh + 1) * NCH_SZ)
                    o_ps = ops_pool.tile([P, NCH_SZ], F32, tag="ops")
                    if DR:
                        for cb in range(0, CB, 2):
                            nc.tensor.matmul(
                                o_ps[:],
                                PT[:, cb : cb + 2, ob * P : (ob + 1) * P],
                                xfc[:, cb : cb + 2, nsl],
                                start=(cb == 0),
                                stop=(cb + 2 >= CB),
                                perf_mode=mybir.MatmulPerfMode.DoubleRow,
                            )
                    else:
                        for cb in range(CB):
                            nc.tensor.matmul(
                                o_ps[:],
                                PT[:, cb, ob * P : (ob + 1) * P],
                                xfc[:, cb, nsl],
                                start=(cb == 0),
                                stop=(cb == CB - 1),
                            )
                    yt = yt_pool.tile([P, NCH_SZ], F32, tag="yt")
                    nc.vector.scalar_tensor_tensor(
                        out=yt[:],
                        in0=o_ps[:],
                        scalar=beta[:, ob : ob + 1],
                        in1=xf[:, ob, nsl],
                        op0=mybir.AluOpType.mult,
                        op1=mybir.AluOpType.add,
                    )
                    nc.sync.dma_start(y[b, ob * P : (ob + 1) * P, nsl], yt[:])

    nc.finalize()
    return nc


def kernel(x: np.ndarray, gamma: np.ndarray) -> np.ndarray:
    global LAST_EXEC_TIME_NS, LAST_TRACE, LAST_PROFILE_JSON
    from concourse.bass_utils import run_bass_kernel_spmd

    assert x.shape == (B, C, H, W), x.shape
    x = np.ascontiguousarray(x, dtype=np.float32)
    gamma = np.ascontiguousarray(gamma, dtype=np.float32).reshape(1)

    name = MM_DT_NAME
    if name not in _CACHE:
        _CACHE[name] = _build(name)
    nc = _CACHE[name]

    xs = x.reshape(N_CORES, BPC, C, N)
    in_maps = [{"x": xs[i], "gamma": gamma} for i in range(N_CORES)]
    trace = os.environ.get("CAM_TRACE", "0") == "1"
    kwargs = {}
    if trace:
        tmpdir = f"/tmp/cam_trace_{name}"
        os.makedirs(tmpdir, exist_ok=True)
        kwargs["tmpdir"] = tmpdir
    res = run_bass_kernel_spmd(
        nc, in_maps, core_ids=list(range(N_CORES)), trace=trace, **kwargs
    )
    LAST_EXEC_TIME_NS = res.exec_time_ns
    LAST_TRACE = res.instructions_and_trace
    LAST_PROFILE_JSON = res.profile_json
    out = np.concatenate([res.results[i]["y"] for i in range(N_CORES)], axis=0)
    return out.reshape(B, C, H, W)


# revision 6
# speedup vs baseline: 1.0986x; 1.0986x over previous
"""nn_CAM_Module kernel for 8 Trainium2 NeuronCores (Bass/Tile).

Contract: kernel(**inputs) takes the FULL inputs (x: [16, 512, 64, 64] fp32,
gamma: [1] fp32) and returns the FULL output, sharding batch B=16 across the
8 cores (2 samples per core, gamma replicated) — per the data-parallel
sharding: every op is a per-sample bmm, no cross-core communication.

Per-sample computation (C=512 channels, N=H*W=4096):
  energy = xf @ xf.T                          (C,C), contraction over N on PE
  m_i    = min_j energy[i,j]                  (softmax(max-e) == softmax(m-e))
  P_ij   = exp(m_i - energy_ij), S_i = sum_j  (ACT, fused row-sum)
  out    = diag(1/S) @ (P @ xf)               (PE; P^T tiles via PE transpose)
  y      = gamma * out + x                    (fused DVE mult-add)

Layouts per core (P=128 partitions):
  xf   [128, 4, 4096] fp32   channel blocks on partitions (DMA from DRAM)
  xfc  [128, 4, 4096] mm_dt  low-precision cast (matmul operand)
  xfT  [128, 32, 512] mm_dt  spatial chunks on partitions (PE transposes)
  Pmat [128, 4, 512]  mm_dt  attention numerator, rows i
  PT   [128, 4, 512]  mm_dt  P^T tiles (PE transposes), matmul2 stationary
"""

import os
from contextlib import ExitStack

import numpy as np

B, C, H, W = 16, 512, 64, 64
N = H * W
N_CORES = 8
BPC = B // N_CORES
P = 128

MM_DT_NAME = os.environ.get("CAM_MM_DT", "bf16")

LAST_EXEC_TIME_NS = None
LAST_TRACE = None
LAST_PROFILE_JSON = None
_CACHE = {}


def _build(mm_dt_name):
    import concourse.mybir as mybir
    import concourse.tile as tile
    from concourse import bacc
    from concourse.masks import make_identity

    F32 = mybir.dt.float32
    mm_dt = {
        "bf16": mybir.dt.bfloat16,
        "fp8": mybir.dt.float8e4,
        "f32": F32,
    }[mm_dt_name]
    DR = mm_dt in (mybir.dt.float8e4, mybir.dt.float8e5)

    CB = C // P          # 4 channel blocks
    KB = N // P          # 32 spatial chunks
    NCH_SZ = 512
    NCH = N // NCH_SZ    # 8 output column chunks

    nc = bacc.Bacc(None, target_bir_lowering=False, debug=False)
    x = nc.dram_tensor("x", [BPC, C, N], F32, kind="ExternalInput")
    gamma = nc.dram_tensor("gamma", [1], F32, kind="ExternalInput")
    y = nc.dram_tensor("y", [BPC, C, N], F32, kind="ExternalOutput")

    with ExitStack() as ctx:
        tc = ctx.enter_context(tile.TileContext(nc))
        singles = ctx.enter_context(tc.tile_pool(name="singles", bufs=1))
        xf_pool = ctx.enter_context(tc.tile_pool(name="xf", bufs=1))
        xfc_pool = ctx.enter_context(tc.tile_pool(name="xfc", bufs=1))
        xfT_pool = ctx.enter_context(tc.tile_pool(name="xfT", bufs=1))
        pmat_pool = ctx.enter_context(tc.tile_pool(name="pmat", bufs=2))
        pt_pool = ctx.enter_context(tc.tile_pool(name="pt", bufs=2))
        small = ctx.enter_context(tc.tile_pool(name="small", bufs=16))
        yt_pool = ctx.enter_context(tc.tile_pool(name="yt", bufs=4))
        eps_pool = ctx.enter_context(tc.tile_pool(name="eps", bufs=2, space="PSUM"))
        tps_pool = ctx.enter_context(tc.tile_pool(name="tps", bufs=3, space="PSUM"))
        ops_pool = ctx.enter_context(tc.tile_pool(name="ops", bufs=3, space="PSUM"))

        ident = singles.tile([P, P], mm_dt)
        make_identity(nc, ident)
        gamma_sb = singles.tile([P, 1], F32)
        nc.sync.dma_start(gamma_sb[:], gamma[:].to_broadcast((P, 1)))

        for b in range(BPC):
            xf = xf_pool.tile([P, CB, N], F32, tag="xf")
            for cb in range(CB):
                nc.sync.dma_start(xf[:, cb, :], x[b, cb * P : (cb + 1) * P, :])

            xfc = xfc_pool.tile([P, CB, N], mm_dt, tag="xfc")
            for cb in range(CB):
                if cb % 2 == 0:
                    nc.vector.tensor_copy(out=xfc[:, cb, :], in_=xf[:, cb, :])
                else:
                    nc.scalar.copy(out=xfc[:, cb, :], in_=xf[:, cb, :])

            # fp8 PE-transpose writes PSUM with element step 2 (16-bit write
            # packing): stage into a 2x-strided PSUM view, copy back strided.
            TW = 2 if DR else 1

            def tps_views(tps):
                if TW == 1:
                    return tps, tps
                v = tps[:].rearrange("p cb (n t) -> p cb n t", t=TW)[:, :, :, 0]
                return v, v

            xfT = xfT_pool.tile([P, KB, C], mm_dt, tag="xfT")
            for k in range(KB):
                tps = tps_pool.tile([P, CB, P * TW], mm_dt, tag="tps")
                wv, rv = tps_views(tps)
                for cb in range(CB):
                    nc.tensor.transpose(
                        wv[:, cb, :], xfc[:, cb, k * P : (k + 1) * P], ident
                    )
                dst = xfT[:, k, :].rearrange("p (cb n) -> p cb n", n=P)
                if k % 2 == 0:
                    nc.vector.tensor_copy(out=dst, in_=rv)
                else:
                    nc.scalar.copy(out=dst, in_=rv)

            Pmat = pmat_pool.tile([P, CB, C], mm_dt, tag="pmat")
            rS = small.tile([P, CB], F32, tag="rS")
            for cb in range(CB):
                e_ps = eps_pool.tile([P, C], F32, tag="eps")
                if DR:
                    for k in range(0, KB, 2):
                        nc.tensor.matmul(
                            e_ps[:],
                            xfT[:, k : k + 2, cb * P : (cb + 1) * P],
                            xfT[:, k : k + 2, :],
                            start=(k == 0),
                            stop=(k + 2 >= KB),
                            perf_mode=mybir.MatmulPerfMode.DoubleRow,
                        )
                else:
                    for k in range(KB):
                        nc.tensor.matmul(
                            e_ps[:],
                            xfT[:, k, cb * P : (cb + 1) * P],
                            xfT[:, k, :],
                            start=(k == 0),
                            stop=(k == KB - 1),
                        )
                m = small.tile([P, 1], F32, tag="m")
                nc.vector.tensor_reduce(
                    out=m[:], in_=e_ps[:], axis=mybir.AxisListType.X,
                    op=mybir.AluOpType.min,
                )
                S = small.tile([P, 1], F32, tag="S")
                nc.scalar.activation(
                    out=Pmat[:, cb, :],
                    in_=e_ps[:],
                    func=mybir.ActivationFunctionType.Exp,
                    bias=m[:],
                    scale=-1.0,
                    accum_out=S[:],
                )
                nc.vector.reciprocal(out=rS[:, cb : cb + 1], in_=S[:])

            beta = small.tile([P, CB], F32, tag="beta")
            nc.vector.tensor_tensor(
                out=beta[:],
                in0=rS[:],
                in1=gamma_sb[:].to_broadcast((P, CB)),
                op=mybir.AluOpType.mult,
            )

            PT = pt_pool.tile([P, CB, C], mm_dt, tag="pt")
            for cb in range(CB):
                tps = tps_pool.tile([P, CB, P * TW], mm_dt, tag="tps")
                wv, rv = tps_views(tps)
                for ob in range(CB):
                    nc.tensor.transpose(
                        wv[:, ob, :], Pmat[:, ob, cb * P : (cb + 1) * P], ident
                    )
                dst = PT[:, cb, :].rearrange("p (ob n) -> p ob n", n=P)
                if cb % 2 == 0:
                    nc.vector.tensor_copy(out=dst, in_=rv)
                else:
                    nc.scalar.copy(out=dst, in_=rv)

            for ob in range(CB):
                for nh in range(NCH):
                    nsl = slice(nh * NCH_SZ, (nh + 1) * NCH_SZ)
                    o_ps = ops_pool.tile([P, NCH_SZ], F32, tag="ops")
                    if DR:
                        for cb in range(0, CB, 2):
                            nc.tensor.matmul(
                                o_ps[:],
                                PT[:, cb : cb + 2, ob * P : (ob + 1) * P],
                                xfc[:, cb : cb + 2, nsl],
                                start=(cb == 0),
                                stop=(cb + 2 >= CB),
                                perf_mode=mybir.MatmulPerfMode.DoubleRow,
                            )
                    else:
                        for cb in range(CB):
                            nc.tensor.matmul(
                                o_ps[:],
                                PT[:, cb, ob * P : (ob + 1) * P],
                                xfc[:, cb, nsl],
                                start=(cb == 0),
                                stop=(cb == CB - 1),
                            )
                    yt = yt_pool.tile([P, NCH_SZ], F32, tag="yt")
                    nc.vector.scalar_tensor_tensor(
                        out=yt[:],
                        in0=o_ps[:],
                        scalar=beta[:, ob : ob + 1],
                        in1=xf[:, ob, nsl],
                        op0=mybir.AluOpType.mult,
                        op1=mybir.AluOpType.add,
                    )
                    nc.sync.dma_start(y[b, ob * P : (ob + 1) * P, nsl], yt[:])

    nc.finalize()
    return nc


def kernel(x: np.ndarray, gamma: np.ndarray) -> np.ndarray:
    global LAST_EXEC_TIME_NS, LAST_TRACE, LAST_PROFILE_JSON
    from concourse.bass_utils import run_bass_kernel_spmd

    assert x.shape == (B, C, H, W), x.shape
    x = np.ascontiguousarray(x, dtype=np.float32)
    gamma = np.ascontiguousarray(gamma, dtype=np.float32).reshape(1)

    name = MM_DT_NAME
    if name not in _CACHE:
        _CACHE[name] = _build(name)
    nc = _CACHE[name]

    xs = x.reshape(N_CORES, BPC, C, N)
    in_maps = [{"x": xs[i], "gamma": gamma} for i in range(N_CORES)]
    trace = os.environ.get("CAM_TRACE", "0") == "1"
    kwargs = {}
    if trace:
        tmpdir = f"/tmp/cam_trace_{name}"
        os.makedirs(tmpdir, exist_ok=True)
        kwargs["tmpdir"] = tmpdir
    res = run_bass_kernel_spmd(
        nc, in_maps, core_ids=list(range(N_CORES)), trace=trace, **kwargs
    )
    LAST_EXEC_TIME_NS = res.exec_time_ns
    LAST_TRACE = res.instructions_and_trace
    LAST_PROFILE_JSON = res.profile_json
    out = np.concatenate([res.results[i]["y"] for i in range(N_CORES)], axis=0)
    return out.reshape(B, C, H, W)


# revision 15
# speedup vs baseline: 1.3701x; 1.2471x over previous
"""nn_CAM_Module kernel for 8 Trainium2 NeuronCores (Bass/Tile).

Contract: kernel(**inputs) takes the FULL inputs (x: [16, 512, 64, 64] fp32,
gamma: [1] fp32) and returns the FULL output, sharding batch B=16 across the
8 cores (2 samples per core, gamma replicated) — per the data-parallel
sharding: every op is a per-sample bmm, no cross-core communication.

Per-sample computation (C=512 channels, N=H*W=4096):
  energy = xf @ xf.T                          (C,C), contraction over N on PE
  m_i    = min_j energy[i,j]                  (softmax(max-e) == softmax(m-e))
  P_ij   = exp(m_i - energy_ij), S_i = sum_j  (ACT, fused row-sum)
  out    = diag(1/S) @ (P @ xf)               (PE; P^T tiles via PE transpose)
  y      = gamma * out + x                    (fused DVE mult-add)

Layouts per core (P=128 partitions):
  xf   [128, 4, 4096] fp32   channel blocks on partitions (DMA from DRAM)
  xfc  [128, 4, 4096] mm_dt  low-precision cast (matmul operand)
  xfT  [128, 32, 512] mm_dt  spatial chunks on partitions (PE transposes)
  Pmat [128, 4, 512]  mm_dt  attention numerator, rows i
  PT   [128, 4, 512]  mm_dt  P^T tiles (PE transposes), matmul2 stationary
"""

import os
from contextlib import ExitStack

import numpy as np

B, C, H, W = 16, 512, 64, 64
N = H * W
N_CORES = 8
BPC = B // N_CORES
P = 128

MM_DT_NAME = os.environ.get("CAM_MM_DT", "bf16")

LAST_EXEC_TIME_NS = None
LAST_TRACE = None
LAST_PROFILE_JSON = None
_CACHE = {}


def _build(mm_dt_name):
    import concourse.mybir as mybir
    import concourse.tile as tile
    from concourse import bacc
    from concourse.masks import make_identity

    F32 = mybir.dt.float32
    mm_dt = {
        "bf16": mybir.dt.bfloat16,
        "fp8": mybir.dt.float8e4,
        "f32": F32,
    }[mm_dt_name]
    DR = mm_dt in (mybir.dt.float8e4, mybir.dt.float8e5)

    CB = C // P          # 4 channel blocks
    KB = N // P          # 32 spatial chunks
    NCH_SZ = 512
    NCH = N // NCH_SZ    # 8 output column chunks

    nc = bacc.Bacc(None, target_bir_lowering=False, debug=False)
    x = nc.dram_tensor("x", [BPC, C, N], F32, kind="ExternalInput")
    gamma = nc.dram_tensor("gamma", [1], F32, kind="ExternalInput")
    y = nc.dram_tensor("y", [BPC, C, N], F32, kind="ExternalOutput")

    with ExitStack() as ctx:
        tc = ctx.enter_context(tile.TileContext(nc))
        singles = ctx.enter_context(tc.tile_pool(name="singles", bufs=1))
        xf_pool = ctx.enter_context(tc.tile_pool(name="xf", bufs=10))
        xfc_pool = ctx.enter_context(tc.tile_pool(name="xfc", bufs=10))
        xfT_pool = ctx.enter_context(tc.tile_pool(name="xfT", bufs=2))
        pmat_pool = ctx.enter_context(tc.tile_pool(name="pmat", bufs=2))
        pt_pool = ctx.enter_context(tc.tile_pool(name="pt", bufs=2))
        small = ctx.enter_context(tc.tile_pool(name="small", bufs=16))
        yt_pool = ctx.enter_context(tc.tile_pool(name="yt", bufs=4))
        eps_pool = ctx.enter_context(tc.tile_pool(name="eps", bufs=2, space="PSUM"))
        tps_pool = ctx.enter_context(tc.tile_pool(name="tps", bufs=3, space="PSUM"))
        ops_pool = ctx.enter_context(tc.tile_pool(name="ops", bufs=3, space="PSUM"))

        ident = singles.tile([P, P], mm_dt)
        make_identity(nc, ident)
        gamma_sb = singles.tile([P, 1], F32)
        nc.sync.dma_start(gamma_sb[:], gamma[:].to_broadcast((P, 1)))

        # fp8 PE-transpose writes PSUM with element step 2 (16-bit write
        # packing): stage into a 2x-strided PSUM view, copy back strided.
        TW = 2 if DR else 1
        KPC = NCH_SZ // P  # transposes-k per n-chunk

        def tps_views(tps):
            if TW == 1:
                return tps, tps
            v = tps[:].rearrange("p cb (n t) -> p cb n t", t=TW)[:, :, :, 0]
            return v, v

        for b in range(BPC):
            xv = x[b].rearrange("(cb p) n -> p cb n", p=P)
            # n-chunked load/cast/transpose pipeline: first PE work starts
            # after one 1MB chunk, and chunk tiles release progressively so
            # the next sample's loads overlap this sample's tail.
            xf_ch = []
            xfc_ch = []
            xfT = xfT_pool.tile([P, KB, C], mm_dt, tag="xfT")
            for ch in range(NCH):
                nsl = slice(ch * NCH_SZ, (ch + 1) * NCH_SZ)
                xfch = xf_pool.tile([P, CB, NCH_SZ], F32, tag="xf")
                nc.sync.dma_start(xfch[:], xv[:, :, nsl])
                xfcch = xfc_pool.tile([P, CB, NCH_SZ], mm_dt, tag="xfc")
                # fine-grained per-cb casts so the first transposes start
                # right after the first sub-cast, split across DVE/ACT
                for cb in range(CB):
                    if (ch * CB + cb) % 2 == 0:
                        nc.vector.tensor_copy(out=xfcch[:, cb, :], in_=xfch[:, cb, :])
                    else:
                        nc.scalar.copy(out=xfcch[:, cb, :], in_=xfch[:, cb, :])
                xf_ch.append(xfch)
                xfc_ch.append(xfcch)
                for kk in range(KPC):
                    k = ch * KPC + kk
                    tps = tps_pool.tile([P, CB, P * TW], mm_dt, tag="tps")
                    wv, rv = tps_views(tps)
                    for cb in range(CB):
                        nc.tensor.transpose(
                            wv[:, cb, :], xfcch[:, cb, kk * P : (kk + 1) * P], ident
                        )
                    dst = xfT[:, k, :].rearrange("p (cb n) -> p cb n", n=P)
                    if k % 2 == 0:
                        nc.vector.tensor_copy(out=dst, in_=rv)
                    else:
                        nc.scalar.copy(out=dst, in_=rv)

            Pmat = pmat_pool.tile([P, CB, C], mm_dt, tag="pmat")
            rS = small.tile([P, CB], F32, tag="rS")
            for cb in range(CB):
                e_ps = eps_pool.tile([P, C], F32, tag="eps")
                if DR:
                    for k in range(0, KB, 2):
                        nc.tensor.matmul(
                            e_ps[:],
                            xfT[:, k : k + 2, cb * P : (cb + 1) * P],
                            xfT[:, k : k + 2, :],
                            start=(k == 0),
                            stop=(k + 2 >= KB),
                            perf_mode=mybir.MatmulPerfMode.DoubleRow,
                        )
                else:
                    for k in range(KB):
                        nc.tensor.matmul(
                            e_ps[:],
                            xfT[:, k, cb * P : (cb + 1) * P],
                            xfT[:, k, :],
                            start=(k == 0),
                            stop=(k == KB - 1),
                        )
                m = small.tile([P, 1], F32, tag="m")
                nc.vector.tensor_reduce(
                    out=m[:], in_=e_ps[:], axis=mybir.AxisListType.X,
                    op=mybir.AluOpType.min,
                )
                S = small.tile([P, 1], F32, tag="S")
                nc.scalar.activation(
                    out=Pmat[:, cb, :],
                    in_=e_ps[:],
                    func=mybir.ActivationFunctionType.Exp,
                    bias=m[:],
                    scale=-1.0,
                    accum_out=S[:],
                )
                nc.vector.reciprocal(out=rS[:, cb : cb + 1], in_=S[:])

            beta = small.tile([P, CB], F32, tag="beta")
            nc.vector.tensor_tensor(
                out=beta[:],
                in0=rS[:],
                in1=gamma_sb[:].to_broadcast((P, CB)),
                op=mybir.AluOpType.mult,
            )

            # PT transposes grouped by source row-block ob so each group can
            # start as soon as exp(ob) lands (no wait for all four exps).
            PT = pt_pool.tile([P, CB, C], mm_dt, tag="pt")
            for ob in range(CB):
                tps = tps_pool.tile([P, CB, P * TW], mm_dt, tag="tps")
                wv, rv = tps_views(tps)
                for cb in range(CB):
                    nc.tensor.transpose(
                        wv[:, cb, :], Pmat[:, ob, cb * P : (cb + 1) * P], ident
                    )
                dst = PT[:, :, ob * P : (ob + 1) * P]
                if ob % 2 == 0:
                    nc.vector.tensor_copy(out=dst, in_=rv)
                else:
                    nc.scalar.copy(out=dst, in_=rv)

            # nh-major so each n-chunk's xf/xfc tiles release early for the
            # next sample's prefetch; y written as one 1MB DMA per n-chunk.
            yv = y[b].rearrange("(ob p) n -> p ob n", p=P)
            for nh in range(NCH):
                nsl = slice(nh * NCH_SZ, (nh + 1) * NCH_SZ)
                yt = yt_pool.tile([P, CB, NCH_SZ], F32, tag="yt")
                for ob in range(CB):
                    o_ps = ops_pool.tile([P, NCH_SZ], F32, tag="ops")
                    if DR:
                        for cb in range(0, CB, 2):
                            nc.tensor.matmul(
                                o_ps[:],
                                PT[:, cb : cb + 2, ob * P : (ob + 1) * P],
                                xfc_ch[nh][:, cb : cb + 2, :],
                                start=(cb == 0),
                                stop=(cb + 2 >= CB),
                                perf_mode=mybir.MatmulPerfMode.DoubleRow,
                            )
                    else:
                        for cb in range(CB):
                            nc.tensor.matmul(
                                o_ps[:],
                                PT[:, cb, ob * P : (ob + 1) * P],
                                xfc_ch[nh][:, cb, :],
                                start=(cb == 0),
                                stop=(cb == CB - 1),
                            )
                    nc.vector.scalar_tensor_tensor(
                        out=yt[:, ob, :],
                        in0=o_ps[:],
                        scalar=beta[:, ob : ob + 1],
                        in1=xf_ch[nh][:, ob, :],
                        op0=mybir.AluOpType.mult,
                        op1=mybir.AluOpType.add,
                    )
                nc.sync.dma_start(yv[:, :, nsl], yt[:])

    nc.finalize()
    return nc


def kernel(x: np.ndarray, gamma: np.ndarray) -> np.ndarray:
    global LAST_EXEC_TIME_NS, LAST_TRACE, LAST_PROFILE_JSON
    from concourse.bass_utils import run_bass_kernel_spmd

    assert x.shape == (B, C, H, W), x.shape
    x = np.ascontiguousarray(x, dtype=np.float32)
    gamma = np.ascontiguousarray(gamma, dtype=np.float32).reshape(1)

    name = MM_DT_NAME
    if name not in _CACHE:
        _CACHE[name] = _build(name)
    nc = _CACHE[name]

    xs = x.reshape(N_CORES, BPC, C, N)
    in_maps = [{"x": xs[i], "gamma": gamma} for i in range(N_CORES)]
    trace = os.environ.get("CAM_TRACE", "0") == "1"
    kwargs = {}
    if trace:
        import tempfile

        tmpdir = tempfile.mkdtemp(prefix=f"cam_trace_{name}_")
        try:
            os.unlink(f"/tmp/cam_trace_{name}")
        except OSError:
            pass
        os.symlink(tmpdir, f"/tmp/cam_trace_{name}")
        kwargs["tmpdir"] = tmpdir
    res = run_bass_kernel_spmd(
        nc, in_maps, core_ids=list(range(N_CORES)), trace=trace, **kwargs
    )
    LAST_EXEC_TIME_NS = res.exec_time_ns
    LAST_TRACE = res.instructions_and_trace
    LAST_PROFILE_JSON = res.profile_json
    out = np.concatenate([res.results[i]["y"] for i in range(N_CORES)], axis=0)
    return out.reshape(B, C, H, W)


# revision 18
# speedup vs baseline: 1.3712x; 1.0008x over previous
"""nn_CAM_Module kernel for 8 Trainium2 NeuronCores (Bass/Tile).

Contract: kernel(**inputs) takes the FULL inputs (x: [16, 512, 64, 64] fp32,
gamma: [1] fp32) and returns the FULL output, sharding batch B=16 across the
8 cores (2 samples per core, gamma replicated) — per the data-parallel
sharding: every op is a per-sample bmm, no cross-core communication.

Per-sample computation (C=512 channels, N=H*W=4096):
  energy = xf @ xf.T                          (C,C), contraction over N on PE
  m_i    = min_j energy[i,j]                  (softmax(max-e) == softmax(m-e))
  P_ij   = exp(m_i - energy_ij), S_i = sum_j  (ACT, fused row-sum)
  out    = diag(1/S) @ (P @ xf)               (PE; P^T tiles via PE transpose)
  y      = gamma * out + x                    (fused DVE mult-add)

Layouts per core (P=128 partitions):
  xf   [128, 4, 4096] fp32   channel blocks on partitions (DMA from DRAM)
  xfc  [128, 4, 4096] mm_dt  low-precision cast (matmul operand)
  xfT  [128, 32, 512] mm_dt  spatial chunks on partitions (PE transposes)
  Pmat [128, 4, 512]  mm_dt  attention numerator, rows i
  PT   [128, 4, 512]  mm_dt  P^T tiles (PE transposes), matmul2 stationary
"""

import os
from contextlib import ExitStack

import numpy as np

B, C, H, W = 16, 512, 64, 64
N = H * W
N_CORES = 8
BPC = B // N_CORES
P = 128

MM_DT_NAME = os.environ.get("CAM_MM_DT", "bf16")

LAST_EXEC_TIME_NS = None
LAST_TRACE = None
LAST_PROFILE_JSON = None
_CACHE = {}


def _build(mm_dt_name):
    import concourse.mybir as mybir
    import concourse.tile as tile
    from concourse import bacc
    from concourse.masks import make_identity

    F32 = mybir.dt.float32
    mm_dt = {
        "bf16": mybir.dt.bfloat16,
        "fp8": mybir.dt.float8e4,
        "f32": F32,
    }[mm_dt_name]
    DR = mm_dt in (mybir.dt.float8e4, mybir.dt.float8e5)

    CB = C // P          # 4 channel blocks
    KB = N // P          # 32 spatial chunks
    NCH_SZ = 512
    NCH = N // NCH_SZ    # 8 output column chunks

    nc = bacc.Bacc(None, target_bir_lowering=False, debug=False)
    x = nc.dram_tensor("x", [BPC, C, N], F32, kind="ExternalInput")
    gamma = nc.dram_tensor("gamma", [1], F32, kind="ExternalInput")
    y = nc.dram_tensor("y", [BPC, C, N], F32, kind="ExternalOutput")

    with ExitStack() as ctx:
        tc = ctx.enter_context(tile.TileContext(nc))
        singles = ctx.enter_context(tc.tile_pool(name="singles", bufs=1))
        xf_pool = ctx.enter_context(tc.tile_pool(name="xf", bufs=12))
        xfc_pool = ctx.enter_context(tc.tile_pool(name="xfc", bufs=12))
        xfT_pool = ctx.enter_context(tc.tile_pool(name="xfT", bufs=2))
        pmat_pool = ctx.enter_context(tc.tile_pool(name="pmat", bufs=2))
        pt_pool = ctx.enter_context(tc.tile_pool(name="pt", bufs=2))
        small = ctx.enter_context(tc.tile_pool(name="small", bufs=16))
        yt_pool = ctx.enter_context(tc.tile_pool(name="yt", bufs=2))
        eps_pool = ctx.enter_context(tc.tile_pool(name="eps", bufs=2, space="PSUM"))
        tps_pool = ctx.enter_context(tc.tile_pool(name="tps", bufs=3, space="PSUM"))
        ops_pool = ctx.enter_context(tc.tile_pool(name="ops", bufs=3, space="PSUM"))

        ident = singles.tile([P, P], mm_dt)
        make_identity(nc, ident)
        gamma_sb = singles.tile([P, 1], F32)
        nc.sync.dma_start(gamma_sb[:], gamma[:].to_broadcast((P, 1)))

        # fp8 PE-transpose writes PSUM with element step 2 (16-bit write
        # packing): stage into a 2x-strided PSUM view, copy back strided.
        TW = 2 if DR else 1
        KPC = NCH_SZ // P  # transposes-k per n-chunk

        def tps_views(tps):
            if TW == 1:
                return tps, tps
            v = tps[:].rearrange("p cb (n t) -> p cb n t", t=TW)[:, :, :, 0]
            return v, v

        for b in range(BPC):
            xv = x[b].rearrange("(cb p) n -> p cb n", p=P)
            # n-chunked load/cast/transpose pipeline: first PE work starts
            # after one 1MB chunk, and chunk tiles release progressively so
            # the next sample's loads overlap this sample's tail.
            xf_ch = []
            xfc_ch = []
            xfT = xfT_pool.tile([P, KB, C], mm_dt, tag="xfT")
            for ch in range(NCH):
                nsl = slice(ch * NCH_SZ, (ch + 1) * NCH_SZ)
                xfch = xf_pool.tile([P, CB, NCH_SZ], F32, tag="xf")
                nc.sync.dma_start(xfch[:], xv[:, :, nsl])
                xfcch = xfc_pool.tile([P, CB, NCH_SZ], mm_dt, tag="xfc")
                # fine-grained per-cb casts so the first transposes start
                # right after the first sub-cast, split across DVE/ACT
                for cb in range(CB):
                    if (ch * CB + cb) % 2 == 0:
                        nc.vector.tensor_copy(out=xfcch[:, cb, :], in_=xfch[:, cb, :])
                    else:
                        nc.scalar.copy(out=xfcch[:, cb, :], in_=xfch[:, cb, :])
                xf_ch.append(xfch)
                xfc_ch.append(xfcch)
                # two k-groups share one PSUM bank: 8 transposes, one copy
                for kk in range(0, KPC, 2):
                    k = ch * KPC + kk
                    tps = tps_pool.tile([P, 2, CB, P * TW], mm_dt, tag="tps")
                    if TW == 1:
                        wv = tps[:]
                    else:
                        wv = tps[:].rearrange("p u cb (n t) -> p u cb n t", t=TW)[
                            :, :, :, :, 0
                        ]
                    for u in range(2):
                        for cb in range(CB):
                            nc.tensor.transpose(
                                wv[:, u, cb, :],
                                xfcch[:, cb, (kk + u) * P : (kk + u + 1) * P],
                                ident,
                            )
                    dst = xfT[:, k : k + 2, :].rearrange(
                        "p u (cb n) -> p u cb n", n=P
                    )
                    if kk % 4 == 0:
                        nc.vector.tensor_copy(out=dst, in_=wv)
                    else:
                        nc.scalar.copy(out=dst, in_=wv)

            Pmat = pmat_pool.tile([P, CB, C], mm_dt, tag="pmat")
            rS = small.tile([P, CB], F32, tag="rS")
            for cb in range(CB):
                e_ps = eps_pool.tile([P, C], F32, tag="eps")
                if DR:
                    for k in range(0, KB, 2):
                        nc.tensor.matmul(
                            e_ps[:],
                            xfT[:, k : k + 2, cb * P : (cb + 1) * P],
                            xfT[:, k : k + 2, :],
                            start=(k == 0),
                            stop=(k + 2 >= KB),
                            perf_mode=mybir.MatmulPerfMode.DoubleRow,
                        )
                else:
                    for k in range(KB):
                        nc.tensor.matmul(
                            e_ps[:],
                            xfT[:, k, cb * P : (cb + 1) * P],
                            xfT[:, k, :],
                            start=(k == 0),
                            stop=(k == KB - 1),
                        )
                m = small.tile([P, 1], F32, tag="m")
                nc.vector.tensor_reduce(
                    out=m[:], in_=e_ps[:], axis=mybir.AxisListType.X,
                    op=mybir.AluOpType.min,
                )
                S = small.tile([P, 1], F32, tag="S")
                nc.scalar.activation(
                    out=Pmat[:, cb, :],
                    in_=e_ps[:],
                    func=mybir.ActivationFunctionType.Exp,
                    bias=m[:],
                    scale=-1.0,
                    accum_out=S[:],
                )
                nc.vector.reciprocal(out=rS[:, cb : cb + 1], in_=S[:])

            beta = small.tile([P, CB], F32, tag="beta")
            nc.vector.tensor_tensor(
                out=beta[:],
                in0=rS[:],
                in1=gamma_sb[:].to_broadcast((P, CB)),
                op=mybir.AluOpType.mult,
            )

            # PT transposes grouped by source row-block ob so each group can
            # start as soon as exp(ob) lands (no wait for all four exps).
            PT = pt_pool.tile([P, CB, C], mm_dt, tag="pt")
            for ob in range(CB):
                tps = tps_pool.tile([P, CB, P * TW], mm_dt, tag="tps")
                wv, rv = tps_views(tps)
                for cb in range(CB):
                    nc.tensor.transpose(
                        wv[:, cb, :], Pmat[:, ob, cb * P : (cb + 1) * P], ident
                    )
                dst = PT[:, :, ob * P : (ob + 1) * P]
                if ob % 2 == 0:
                    nc.vector.tensor_copy(out=dst, in_=rv)
                else:
                    nc.scalar.copy(out=dst, in_=rv)

            # nh-major so each n-chunk's xf/xfc tiles release early for the
            # next sample's prefetch; y written as one 1MB DMA per n-chunk.
            yv = y[b].rearrange("(ob p) n -> p ob n", p=P)
            for nh in range(NCH):
                nsl = slice(nh * NCH_SZ, (nh + 1) * NCH_SZ)
                yt = yt_pool.tile([P, CB, NCH_SZ], F32, tag="yt")
                for ob in range(CB):
                    o_ps = ops_pool.tile([P, NCH_SZ], F32, tag="ops")
                    if DR:
                        for cb in range(0, CB, 2):
                            nc.tensor.matmul(
                                o_ps[:],
                                PT[:, cb : cb + 2, ob * P : (ob + 1) * P],
                                xfc_ch[nh][:, cb : cb + 2, :],
                                start=(cb == 0),
                                stop=(cb + 2 >= CB),
                                perf_mode=mybir.MatmulPerfMode.DoubleRow,
                            )
                    else:
                        for cb in range(CB):
                            nc.tensor.matmul(
                                o_ps[:],
                                PT[:, cb, ob * P : (ob + 1) * P],
                                xfc_ch[nh][:, cb, :],
                                start=(cb == 0),
                                stop=(cb == CB - 1),
                            )
                    nc.vector.scalar_tensor_tensor(
                        out=yt[:, ob, :],
                        in0=o_ps[:],
                        scalar=beta[:, ob : ob + 1],
                        in1=xf_ch[nh][:, ob, :],
                        op0=mybir.AluOpType.mult,
                        op1=mybir.AluOpType.add,
                    )
                nc.sync.dma_start(yv[:, :, nsl], yt[:])

    nc.finalize()
    return nc


def kernel(x: np.ndarray, gamma: np.ndarray) -> np.ndarray:
    global LAST_EXEC_TIME_NS, LAST_TRACE, LAST_PROFILE_JSON
    from concourse.bass_utils import run_bass_kernel_spmd

    assert x.shape == (B, C, H, W), x.shape
    x = np.ascontiguousarray(x, dtype=np.float32)
    gamma = np.ascontiguousarray(gamma, dtype=np.float32).reshape(1)

    name = MM_DT_NAME
    if name not in _CACHE:
        _CACHE[name] = _build(name)
    nc = _CACHE[name]

    xs = x.reshape(N_CORES, BPC, C, N)
    in_maps = [{"x": xs[i], "gamma": gamma} for i in range(N_CORES)]
    trace = os.environ.get("CAM_TRACE", "0") == "1"
    kwargs = {}
    if trace:
        import tempfile

        tmpdir = tempfile.mkdtemp(prefix=f"cam_trace_{name}_")
        try:
            os.unlink(f"/tmp/cam_trace_{name}")
        except OSError:
            pass
        os.symlink(tmpdir, f"/tmp/cam_trace_{name}")
        kwargs["tmpdir"] = tmpdir
    res = run_bass_kernel_spmd(
        nc, in_maps, core_ids=list(range(N_CORES)), trace=trace, **kwargs
    )
    LAST_EXEC_TIME_NS = res.exec_time_ns
    LAST_TRACE = res.instructions_and_trace
    LAST_PROFILE_JSON = res.profile_json
    out = np.concatenate([res.results[i]["y"] for i in range(N_CORES)], axis=0)
    return out.reshape(B, C, H, W)
